# revision 1
# baseline (speedup 1.0000x reference)
"""GATv2 2-layer GNN on 8 Trainium2 NeuronCores (Bass/Tile, edge-parallel).

Sharding: edges sorted by dst node, dst-range sharded across 8 cores
(core k owns dst nodes [1250k, 1250(k+1))), so the per-dst segment
softmax and aggregation are fully core-local. Node-side projections for
layer 1 are computed replicated (xl1 for all nodes; xr1 for the own
slice). Between layers only the 32-wide layer-2 projections are
exchanged with a single AllGather.
"""
import sys
sys.path.insert(0, "/opt/trn_rl_repo")

import numpy as np
import ml_dtypes

import concourse.bass as bass
import concourse.bacc as bacc
import concourse.tile as tile
from concourse import mybir
from concourse.bass_utils import run_bass_kernel_spmd

BF16 = ml_dtypes.bfloat16

N, E, F = 10000, 80000, 128
H1, C1 = 8, 256
D1 = H1 * C1          # 2048
D2 = 32               # layer-2 out (1 head)
NEG = 0.2
M = 8                 # cores
NPC = N // M          # 1250 nodes per core
GN = 125              # dst nodes per group
G = NPC // GN         # 10 groups per core
P = 128

dt = mybir.dt


def _build_program(CH, phases="ABC"):
    """Build the SPMD Bass program. CH = chunks per group (incl. 1 self chunk)."""
    L = G * CH * P  # edge-stream length per core
    nc = bacc.Bacc("TRN2", target_bir_lowering=False, debug=False, num_devices=M)

    # ---- external inputs (per-core data differs only for edge/slice tensors)
    ei = {}
    def EIN(name, shape, dtype):
        ei[name] = nc.dram_tensor(name, list(shape), dtype, kind="ExternalInput")
        return ei[name]

    xT    = EIN("xT",    (P, N),      dt.bfloat16)   # x transposed (replicated)
    xsT   = EIN("xsT",   (P, G * P),  dt.bfloat16)   # own-slice cols, group padded
    wl1   = EIN("wl1",   (F, D1),     dt.bfloat16)
    wr1   = EIN("wr1",   (F, D1),     dt.bfloat16)
    we1   = EIN("we1",   (F, D1),     dt.bfloat16)
    att1r = EIN("att1r", (P, D1),     dt.bfloat16)   # att1 row-replicated
    wl2   = EIN("wl2",   (P, 16 * D2), dt.bfloat16)  # [p, k*32+c] = Wl2[k*128+p, c]
    wr2   = EIN("wr2",   (P, 16 * D2), dt.bfloat16)
    we2   = EIN("we2",   (F, D2),     dt.bfloat16)
    att2r = EIN("att2r", (P, D2),     dt.bfloat16)
    eaT   = EIN("eaT",   (F, L),      dt.bfloat16)   # edge_attr^T, sorted+padded
    eaN   = EIN("eaN",   (L, F),      dt.bfloat16)   # edge_attr row-major
    s01   = EIN("s01",   (L, P),      dt.bfloat16)   # one-hot dst selector
    srci  = EIN("srci",  (L, 1),      dt.int32)      # global src node id
    dstpi = EIN("dstpi", (L, 1),      dt.int32)      # group-padded local dst id
    dstgi = EIN("dstgi", (L, 1),      dt.int32)      # global dst id
    invc  = EIN("invc",  (G * P, 1),  dt.float32)    # 1/max(cnt,1) per dst node

    out = nc.dram_tensor("out", [NPC, D2], dt.float32, kind="ExternalOutput")

    # ---- DRAM scratch
    xl1_tab = nc.dram_tensor("xl1_tab", [N, D1], dt.bfloat16)
    xr1_sl  = nc.dram_tensor("xr1_sl", [G * P, D1], dt.bfloat16)
    xlr2    = nc.dram_tensor("xlr2", [NPC, 2 * D2], dt.float32)
    ag_out  = nc.dram_tensor("ag_out", [N, 2 * D2], dt.float32, addr_space="Shared")
    xl2_tab = nc.dram_tensor("xl2_tab", [N, D2], dt.float32)
    xr2_tab = nc.dram_tensor("xr2_tab", [N, D2], dt.float32)

    AF = mybir.ActivationFunctionType
    ALU = mybir.AluOpType

    with tile.TileContext(nc) as tc:
        with tc.tile_pool(name="consts", bufs=1) as cp:
            xT_sb = cp.tile([P, N], dt.bfloat16)
            nc.sync.dma_start(out=xT_sb[:], in_=xT[:])
            xsT_sb = cp.tile([P, G * P], dt.bfloat16)
            nc.sync.dma_start(out=xsT_sb[:], in_=xsT[:])
            wl1_sb = cp.tile([F, D1], dt.bfloat16)
            nc.sync.dma_start(out=wl1_sb[:], in_=wl1[:])
            wr1_sb = cp.tile([F, D1], dt.bfloat16)
            nc.sync.dma_start(out=wr1_sb[:], in_=wr1[:])
            we1_sb = cp.tile([F, D1], dt.bfloat16)
            nc.sync.dma_start(out=we1_sb[:], in_=we1[:])
            att1_sb = cp.tile([P, D1], dt.bfloat16)
            nc.sync.dma_start(out=att1_sb[:], in_=att1r[:])
            wl2_sb = cp.tile([P, 16 * D2], dt.bfloat16)
            nc.sync.dma_start(out=wl2_sb[:], in_=wl2[:])
            wr2_sb = cp.tile([P, 16 * D2], dt.bfloat16)
            nc.sync.dma_start(out=wr2_sb[:], in_=wr2[:])
            we2_sb = cp.tile([F, D2], dt.bfloat16)
            nc.sync.dma_start(out=we2_sb[:], in_=we2[:])
            att2_sb = cp.tile([P, D2], dt.bfloat16)
            nc.sync.dma_start(out=att2_sb[:], in_=att2r[:])

            # ---------- phase A: node projections ----------
            if "A" in phases:
                with (
                  tc.tile_pool(name="a_ps", bufs=2, space="PSUM") as aps,
                  tc.tile_pool(name="a_sb", bufs=3) as asb,
              ):
                  # xl1 for ALL nodes (replicated compute)
                  for t in range((N + P - 1) // P):
                      mt = min(P, N - t * P)
                      ps = aps.tile([P, D1], dt.float32, tag="ps")
                      for j in range(4):
                          nc.tensor.matmul(
                              out=ps[:mt, j * 512:(j + 1) * 512],
                              lhsT=xT_sb[:, t * P:t * P + mt],
                              rhs=wl1_sb[:, j * 512:(j + 1) * 512],
                              start=True, stop=True,
                          )
                      xsb = asb.tile([P, D1], dt.bfloat16, tag="xsb")
                      nc.scalar.copy(out=xsb[:mt], in_=ps[:mt])
                      nc.sync.dma_start(out=xl1_tab[t * P:t * P + mt, :], in_=xsb[:mt])
                  # xr1 for own slice only
                  for g in range(G):
                      ps = aps.tile([P, D1], dt.float32, tag="ps")
                      for j in range(4):
                          nc.tensor.matmul(
                              out=ps[:GN, j * 512:(j + 1) * 512],
                              lhsT=xsT_sb[:, g * P:g * P + GN],
                              rhs=wr1_sb[:, j * 512:(j + 1) * 512],
                              start=True, stop=True,
                          )
                      xsb = asb.tile([P, D1], dt.bfloat16, tag="xsb")
                      nc.scalar.copy(out=xsb[:GN], in_=ps[:GN])
                      nc.sync.dma_start(out=xr1_sl[g * P:g * P + GN, :], in_=xsb[:GN])

            # ---------- phase B: layer-1 edge pass + layer-2 projections ----------
            if "B" in phases:
                with (
                  tc.tile_pool(name="b_acc", bufs=1, space="PSUM") as accp,   # 5 banks
                  tc.tile_pool(name="b_small", bufs=1, space="PSUM") as smallp,  # 1 bank
                  tc.tile_pool(name="b_eproj", bufs=1, space="PSUM") as eprojp,  # 2 banks
                  tc.tile_pool(name="b_sb", bufs=3) as bsb,
                  tc.tile_pool(name="b_sb2", bufs=2) as bsb2,
                  tc.tile_pool(name="saT", bufs=G) as satp,
              ):
                  saT_tiles = []
                  for g in range(G):
                      acc = accp.tile([P, D1 + 8], dt.float32, tag="acc")
                      selfsum = smallp.tile([P, F], dt.float32, tag="small")
                      invc_t = bsb.tile([P, 1], dt.float32, tag="invc")
                      nc.sync.dma_start(out=invc_t[:], in_=invc[g * P:(g + 1) * P, :])

                      for ch in range(CH):
                          is_self = ch == CH - 1
                          e0 = (g * CH + ch) * P
                          si = bsb.tile([P, 1], dt.int32, tag="si")
                          nc.sync.dma_start(out=si[:], in_=srci[e0:e0 + P, :])
                          di = bsb.tile([P, 1], dt.int32, tag="di")
                          nc.sync.dma_start(out=di[:], in_=dstpi[e0:e0 + P, :])
                          s01_t = bsb.tile([P, P], dt.bfloat16, tag="s01")
                          nc.sync.dma_start(out=s01_t[:], in_=s01[e0:e0 + P, :])
                          xl_g = bsb.tile([P, D1], dt.bfloat16, tag="xl")
                          nc.gpsimd.indirect_dma_start(
                              out=xl_g[:], out_offset=None, in_=xl1_tab[:],
                              in_offset=bass.IndirectOffsetOnAxis(ap=si[:, :1], axis=0))
                          xr_g = bsb.tile([P, D1], dt.bfloat16, tag="xr")
                          nc.gpsimd.indirect_dma_start(
                              out=xr_g[:], out_offset=None, in_=xr1_sl[:],
                              in_offset=bass.IndirectOffsetOnAxis(ap=di[:, :1], axis=0))

                          if not is_self:
                              eaT_t = bsb.tile([F, P], dt.bfloat16, tag="eaT")
                              nc.sync.dma_start(out=eaT_t[:], in_=eaT[:, e0:e0 + P])
                              eaN_t = bsb.tile([P, F], dt.bfloat16, tag="eaN")
                              nc.sync.dma_start(out=eaN_t[:], in_=eaN[e0:e0 + P, :])
                              nc.tensor.matmul(
                                  out=selfsum[:], lhsT=s01_t[:], rhs=eaN_t[:],
                                  start=(ch == 0), stop=(ch == CH - 2))
                              lhs_e = eaT_t
                          else:
                              sattr = bsb.tile([P, F], dt.bfloat16, tag="sattr")
                              nc.vector.tensor_scalar(
                                  out=sattr[:], in0=selfsum[:], scalar1=invc_t[:, :1],
                                  scalar2=None, op0=ALU.mult)
                              saT = satp.tile([P, F], dt.bfloat16, tag="saT")
                              nc.sync.dma_start(out=saT[:], in_=sattr[:], transpose=True)
                              saT_tiles.append(saT)
                              lhs_e = saT

                          logit = bsb.tile([P, H1], dt.float32, tag="logit")
                          for half in range(2):
                              c0 = half * 1024
                              ep = eprojp.tile([P, 1024], dt.float32, tag="eproj")
                              for j in range(2):
                                  nc.tensor.matmul(
                                      out=ep[:, j * 512:(j + 1) * 512],
                                      lhsT=lhs_e[:],
                                      rhs=we1_sb[:, c0 + j * 512:c0 + (j + 1) * 512],
                                      start=True, stop=True)
                              q = bsb2.tile([P, 1024], dt.bfloat16, tag="q")
                              nc.vector.tensor_add(
                                  out=q[:], in0=xl_g[:, c0:c0 + 1024],
                                  in1=xr_g[:, c0:c0 + 1024])
                              q2 = bsb2.tile([P, 1024], dt.bfloat16, tag="q2")
                              nc.vector.tensor_add(out=q2[:], in0=q[:], in1=ep[:])
                              a_t = bsb2.tile([P, 1024], dt.bfloat16, tag="a")
                              nc.scalar.activation(out=a_t[:], in_=q2[:], func=AF.Abs)
                              m_t = bsb2.tile([P, 1024], dt.bfloat16, tag="m")
                              nc.vector.scalar_tensor_tensor(
                                  out=m_t[:], in0=a_t[:], scalar=2.0 / 3.0,
                                  in1=q2[:], op0=ALU.mult, op1=ALU.add)
                              for hh in range(4):
                                  h = half * 4 + hh
                                  scr = bsb2.tile([P, C1], dt.bfloat16, tag="scr")
                                  nc.vector.scalar_tensor_tensor(
                                      out=scr[:],
                                      in0=m_t[:, hh * C1:(hh + 1) * C1],
                                      scalar=1.0,
                                      in1=att1_sb[:, h * C1:(h + 1) * C1],
                                      op0=ALU.mult, op1=ALU.mult,
                                      accum_out=logit[:, h:h + 1])
                          ex = bsb.tile([P, H1], dt.float32, tag="ex")
                          nc.scalar.activation(out=ex[:], in_=logit[:], func=AF.Exp)
                          xls = bsb.tile([P, D1 + 8], dt.bfloat16, tag="xls")
                          for h in range(H1):
                              nc.vector.tensor_scalar(
                                  out=xls[:, h * C1:(h + 1) * C1],
                                  in0=xl_g[:, h * C1:(h + 1) * C1],
                                  scalar1=ex[:, h:h + 1], scalar2=None, op0=ALU.mult)
                          nc.vector.tensor_copy(out=xls[:, D1:D1 + 8], in_=ex[:])
                          for j in range(4):
                              nc.tensor.matmul(
                                  out=acc[:, j * 512:(j + 1) * 512],
                                  lhsT=s01_t[:], rhs=xls[:, j * 512:(j + 1) * 512],
                                  start=(ch == 0), stop=(ch == CH - 1))
                          nc.tensor.matmul(
                              out=acc[:, D1:D1 + 8], lhsT=s01_t[:],
                              rhs=xls[:, D1:D1 + 8],
                              start=(ch == 0), stop=(ch == CH - 1))

                      # -- group finalize: h = relu(acc/denom), layer-2 projections
                      dn_r = bsb.tile([P, H1], dt.float32, tag="dnr")
                      nc.vector.reciprocal(out=dn_r[:], in_=acc[:, D1:D1 + 8])
                      h_sb = bsb.tile([P, D1], dt.bfloat16, tag="hg")
                      for h in range(H1):
                          nc.vector.tensor_scalar(
                              out=h_sb[:, h * C1:(h + 1) * C1],
                              in0=acc[:, h * C1:(h + 1) * C1],
                              scalar1=dn_r[:, h:h + 1], scalar2=0.0,
                              op0=ALU.mult, op1=ALU.max)
                      xl2_ps = smallp.tile([P, D2], dt.float32, tag="small")
                      xr2_ps = eprojp.tile([P, D2], dt.float32, tag="eproj")
                      for kk in range(16):
                          hT = bsb2.tile([P, P], dt.bfloat16, tag="hT")
                          nc.sync.dma_start(out=hT[:], in_=h_sb[:, kk * P:(kk + 1) * P],
                                            transpose=True)
                          nc.tensor.matmul(
                              out=xl2_ps[:], lhsT=hT[:],
                              rhs=wl2_sb[:, kk * D2:(kk + 1) * D2],
                              start=(kk == 0), stop=(kk == 15))
                          nc.tensor.matmul(
                              out=xr2_ps[:], lhsT=hT[:],
                              rhs=wr2_sb[:, kk * D2:(kk + 1) * D2],
                              start=(kk == 0), stop=(kk == 15))
                      xlr2_sb = bsb.tile([P, 2 * D2], dt.float32, tag="xlr2")
                      nc.vector.tensor_copy(out=xlr2_sb[:, :D2], in_=xl2_ps[:])
                      nc.vector.tensor_copy(out=xlr2_sb[:, D2:], in_=xr2_ps[:])
                      nc.sync.dma_start(out=xlr2[g * GN:(g + 1) * GN, :],
                                        in_=xlr2_sb[:GN])

            # ---------- AllGather of layer-2 projections ----------
            if "C" in phases:
              nc.gpsimd.collective_compute(
                  "AllGather", ALU.bypass, replica_groups=[list(range(M))],
                  ins=[xlr2[:]], outs=[ag_out[:]])
              nc.sync.dma_start(out=xl2_tab[:], in_=ag_out[:, :D2])
              nc.sync.dma_start(out=xr2_tab[:], in_=ag_out[:, D2:])

              # ---------- phase C: layer-2 edge pass ----------
              with (
                  tc.tile_pool(name="c_ps", bufs=1, space="PSUM") as cps,
                  tc.tile_pool(name="c_ps2", bufs=2, space="PSUM") as cps2,
                  tc.tile_pool(name="c_sb", bufs=3) as csb,
              ):
                  for g in range(G):
                      acc2 = cps.tile([P, D2 + 1], dt.float32, tag="acc2")
                      for ch in range(CH):
                          is_self = ch == CH - 1
                          e0 = (g * CH + ch) * P
                          si = csb.tile([P, 1], dt.int32, tag="si")
                          nc.sync.dma_start(out=si[:], in_=srci[e0:e0 + P, :])
                          dgi = csb.tile([P, 1], dt.int32, tag="dgi")
                          nc.sync.dma_start(out=dgi[:], in_=dstgi[e0:e0 + P, :])
                          s01_t = csb.tile([P, P], dt.bfloat16, tag="s01")
                          nc.sync.dma_start(out=s01_t[:], in_=s01[e0:e0 + P, :])
                          xl2e = csb.tile([P, D2], dt.float32, tag="xl2e")
                          nc.gpsimd.indirect_dma_start(
                              out=xl2e[:], out_offset=None, in_=xl2_tab[:],
                              in_offset=bass.IndirectOffsetOnAxis(ap=si[:, :1], axis=0))
                          xr2e = csb.tile([P, D2], dt.float32, tag="xr2e")
                          nc.gpsimd.indirect_dma_start(
                              out=xr2e[:], out_offset=None, in_=xr2_tab[:],
                              in_offset=bass.IndirectOffsetOnAxis(ap=dgi[:, :1], axis=0))
                          ep2 = cps2.tile([P, D2], dt.float32, tag="ep2")
                          if not is_self:
                              eaT_t = csb.tile([F, P], dt.bfloat16, tag="eaT")
                              nc.sync.dma_start(out=eaT_t[:], in_=eaT[:, e0:e0 + P])
                              lhs_e = eaT_t
                          else:
                              lhs_e = saT_tiles[g]
                          nc.tensor.matmul(out=ep2[:], lhsT=lhs_e[:], rhs=we2_sb[:],
                                           start=True, stop=True)
                          q2a = csb.tile([P, D2], dt.float32, tag="q2a")
                          nc.vector.tensor_add(out=q2a[:], in0=xl2e[:], in1=xr2e[:])
                          q2b = csb.tile([P, D2], dt.float32, tag="q2b")
                          nc.vector.tensor_add(out=q2b[:], in0=q2a[:], in1=ep2[:])
                          a2 = csb.tile([P, D2], dt.bfloat16, tag="a2")
                          nc.scalar.activation(out=a2[:], in_=q2b[:], func=AF.Abs)
                          m2 = csb.tile([P, D2], dt.bfloat16, tag="m2")
                          nc.vector.scalar_tensor_tensor(
                              out=m2[:], in0=a2[:], scalar=2.0 / 3.0,
                              in1=q2b[:], op0=ALU.mult, op1=ALU.add)
                          lgt2 = csb.tile([P, 1], dt.float32, tag="lgt2")
                          scr2 = csb.tile([P, D2], dt.bfloat16, tag="scr2")
                          nc.vector.scalar_tensor_tensor(
                              out=scr2[:], in0=m2[:], scalar=1.0, in1=att2_sb[:],
                              op0=ALU.mult, op1=ALU.mult,
                              accum_out=lgt2[:, :1])
                          ex2 = csb.tile([P, 1], dt.float32, tag="ex2")
                          nc.scalar.activation(out=ex2[:], in_=lgt2[:], func=AF.Exp)
                          xls2 = csb.tile([P, D2 + 1], dt.bfloat16, tag="xls2")
                          nc.vector.tensor_scalar(
                              out=xls2[:, :D2], in0=xl2e[:], scalar1=ex2[:, :1],
                              scalar2=None, op0=ALU.mult)
                          nc.vector.tensor_copy(out=xls2[:, D2:], in_=ex2[:])
                          nc.tensor.matmul(
                              out=acc2[:], lhsT=s01_t[:], rhs=xls2[:],
                              start=(ch == 0), stop=(ch == CH - 1))
                      d2r = csb.tile([P, 1], dt.float32, tag="d2r")
                      nc.vector.reciprocal(out=d2r[:], in_=acc2[:, D2:D2 + 1])
                      o2 = csb.tile([P, D2], dt.float32, tag="o2")
                      nc.vector.tensor_scalar(
                          out=o2[:], in0=acc2[:, :D2], scalar1=d2r[:, :1],
                          scalar2=0.0, op0=ALU.mult, op1=ALU.max)
                      nc.sync.dma_start(out=out[g * GN:(g + 1) * GN, :], in_=o2[:GN])

    nc.compile()
    return nc


def _prep_inputs(x, edge_index, edge_attr, Wl1, bl1, Wr1, br1, We1, att1, b1,
                 Wl2, bl2, Wr2, br2, We2, att2, b2):
    for b in (bl1, br1, b1, bl2, br2, b2):
        assert not np.any(np.asarray(b)), "nonzero biases not implemented"

    src = np.asarray(edge_index[0], dtype=np.int64)
    dst = np.asarray(edge_index[1], dtype=np.int64)
    ea = np.asarray(edge_attr, dtype=np.float32)
    order = np.argsort(dst, kind="stable")
    s_src, s_dst, s_ea = src[order], dst[order], ea[order]

    # group boundaries: 80 groups of GN dst nodes
    bounds = np.searchsorted(s_dst, np.arange(0, N + GN, GN))
    cnts = np.diff(bounds)  # edges per group (80,)
    G_CH = int(np.max((cnts + P - 1) // P))
    CH = G_CH + 1
    L = G * CH * P

    # per-node incoming counts
    node_cnt = np.bincount(s_dst, minlength=N).astype(np.float32)

    x = np.asarray(x, dtype=np.float32)
    common = {
        "xT": x.T.astype(BF16),
        "wl1": np.asarray(Wl1, np.float32).astype(BF16),
        "wr1": np.asarray(Wr1, np.float32).astype(BF16),
        "we1": np.asarray(We1, np.float32).astype(BF16),
        "att1r": (0.5 * (1 + NEG) * np.tile(
            np.asarray(att1, np.float32).reshape(1, D1), (P, 1))).astype(BF16),
        "wl2": np.asarray(Wl2, np.float32).reshape(16, P, D2)
               .transpose(1, 0, 2).reshape(P, 16 * D2).astype(BF16),
        "wr2": np.asarray(Wr2, np.float32).reshape(16, P, D2)
               .transpose(1, 0, 2).reshape(P, 16 * D2).astype(BF16),
        "we2": np.asarray(We2, np.float32).astype(BF16),
        "att2r": (0.5 * (1 + NEG) * np.tile(
            np.asarray(att2, np.float32).reshape(1, D2), (P, 1))).astype(BF16),
    }

    in_maps = []
    for k in range(M):
        base_node = k * NPC
        eaT_c = np.zeros((L, F), np.float32)   # will transpose at the end
        eaN_c = np.zeros((L, F), np.float32)
        s01_c = np.zeros((L, P), np.float32)
        srci_c = np.zeros((L, 1), np.int32)
        dstpi_c = np.zeros((L, 1), np.int32)
        dstgi_c = np.full((L, 1), base_node, np.int32)
        invc_c = np.ones((G * P, 1), np.float32)
        for g in range(G):
            gb = base_node + g * GN
            lo, hi = bounds[k * G + g], bounds[k * G + g + 1]
            cnt = hi - lo
            assert cnt <= G_CH * P
            o0 = (g * CH) * P  # stream offset of this group's first chunk
            # real edges
            eaT_c[o0:o0 + cnt] = s_ea[lo:hi]
            eaN_c[o0:o0 + cnt] = s_ea[lo:hi]
            dl = (s_dst[lo:hi] - gb).astype(np.int64)  # 0..GN-1
            s01_c[np.arange(o0, o0 + cnt), dl] = 1.0
            srci_c[o0:o0 + cnt, 0] = s_src[lo:hi]
            dstpi_c[o0:o0 + cnt, 0] = g * P + dl
            dstgi_c[o0:o0 + cnt, 0] = s_dst[lo:hi]
            # self chunk (last chunk of the group)
            so = (g * CH + CH - 1) * P
            nn = np.arange(GN)
            s01_c[so + nn, nn] = 1.0
            srci_c[so + nn, 0] = gb + nn
            dstpi_c[so + nn, 0] = g * P + nn
            dstgi_c[so + nn, 0] = gb + nn
            invc_c[g * P + nn, 0] = 1.0 / np.maximum(node_cnt[gb + nn], 1.0)
        im = dict(common)
        im["xsT"] = np.ascontiguousarray(
            np.pad(x[base_node:base_node + NPC].T.reshape(F, G, GN),
                   ((0, 0), (0, 0), (0, P - GN))).reshape(F, G * P)).astype(BF16)
        im["eaT"] = np.ascontiguousarray(eaT_c.T).astype(BF16)
        im["eaN"] = eaN_c.astype(BF16)
        im["s01"] = s01_c.astype(BF16)
        im["srci"] = srci_c
        im["dstpi"] = dstpi_c
        im["dstgi"] = dstgi_c
        im["invc"] = invc_c
        in_maps.append(im)
    return in_maps, CH


_PROG_CACHE = {}


def _get_program(CH, phases="ABC"):
    key = (CH, phases)
    if key not in _PROG_CACHE:
        _PROG_CACHE[key] = _build_program(CH, phases)
    return _PROG_CACHE[key]


def run(inputs, trace=False, tmpdir=None, phases="ABC"):
    in_maps, CH = _prep_inputs(**inputs)
    nc = _get_program(CH, phases)
    res = run_bass_kernel_spmd(nc, in_maps, list(range(M)), trace=trace,
                               tmpdir=tmpdir)
    outp = np.concatenate([res.results[k]["out"] for k in range(M)], axis=0)
    return outp.astype(np.float32), res


def kernel(**inputs):
    outp, _ = run(inputs)
    return outp



# revision 2
# speedup vs baseline: 1.0265x; 1.0265x over previous
"""GATv2 2-layer GNN on 8 Trainium2 NeuronCores (Bass/Tile, edge-parallel v3).

Key structure:
- dst-sorted edges, dst-range sharded (core k owns nodes [1250k,1250(k+1))),
  10 groups x 125 dst nodes, CH chunks of 128 edges (self loops packed on
  host with mean edge_attr).
- Host pre-gathers x[src]^T | x[dst]^T | ea^T per chunk into one stream;
  layer-1 q accumulates fully in PSUM (3 matmul passes), xl extracted by a
  psum->sbuf copy between the passes.
- |att| folded into weight columns (host), columns permuted pos-att-first
  per head: logits = 3D-blockwise reduce of sign * Prelu(q), one scalar
  activation per 512-quarter, one gpsimd TT (sign), one vector reduce.
- alpha scaling of the aggregation undone at group finalize (1/|att| TT).
- scatter matmuls software-pipelined one chunk late; group finalize
  (transpose + layer-2 projection) deferred into the next group's chunks.
- layer 2 identical tricks at width 32; AllGather ships only [N,64] bf16.
"""
import sys
sys.path.insert(0, "/opt/trn_rl_repo")

import numpy as np
import ml_dtypes

import concourse.bass as bass
import concourse.bacc as bacc
import concourse.tile as tile
from concourse import mybir
from concourse.bass_utils import run_bass_kernel_spmd

BF16 = ml_dtypes.bfloat16

N, E, F = 10000, 80000, 128
H1, C1 = 8, 256
D1 = H1 * C1
D2 = 32
NEG = 0.2
M = 8
NPC = N // M
GN = 125
G = NPC // GN
P = 128

dt = mybir.dt
AF = mybir.ActivationFunctionType
ALU = mybir.AluOpType
LRELU = AF.Prelu
FINC = 6          # chunk index within next group at which deferred fin-MMs run
CPRE = 2          # phase-C gather prefetch depth


def _build_program(CH):
    L = G * CH * P
    TC = G * CH
    nc = bacc.Bacc("TRN2", target_bir_lowering=False, debug=False, num_devices=M)

    def EIN(name, shape, dtype):
        return nc.dram_tensor(name, list(shape), dtype, kind="ExternalInput")

    qstream = EIN("qstream", (P, G * CH * 384), dt.bfloat16)
    s01s    = EIN("s01s",    (P, G * CH * P),   dt.bfloat16)
    s01Ts   = EIN("s01Ts",   (P, G * CH * P),   dt.bfloat16)
    srcis   = EIN("srcis",   (L, 1),            dt.int32)
    wl1p    = EIN("wl1p",    (F, D1), dt.bfloat16)
    wr1p    = EIN("wr1p",    (F, D1), dt.bfloat16)
    we1p    = EIN("we1p",    (F, D1), dt.bfloat16)
    srep1   = EIN("srep1",   (P, D1), dt.bfloat16)
    attinv  = EIN("attinv",  (P, D1), dt.float32)
    wlr2    = EIN("wlr2",    (P, 16 * 2 * D2), dt.bfloat16)
    we2p    = EIN("we2p",    (F, D2), dt.bfloat16)
    srep2   = EIN("srep2",   (P, D2), dt.bfloat16)
    attinv2 = EIN("attinv2", (P, D2), dt.float32)
    ident   = EIN("ident",   (P, P), dt.bfloat16)

    out = nc.dram_tensor("out", [NPC, D2], dt.float32, kind="ExternalOutput")
    xlr2_d = nc.dram_tensor("xlr2_d", [NPC, 2 * D2], dt.bfloat16)
    ag_out = nc.dram_tensor("ag_out", [N, 2 * D2], dt.bfloat16, addr_space="Shared")
    xl2t   = nc.dram_tensor("xl2t", [N, D2], dt.bfloat16)

    with tile.TileContext(nc) as tc:
        with tc.tile_pool(name="consts", bufs=1) as cp:
            def CONST(dram, shape, dtype, nm):
                t = cp.tile(shape, dtype, name=nm, tag=nm)
                nc.sync.dma_start(out=t[:], in_=dram[:])
                return t
            wl1_sb = CONST(wl1p, [F, D1], dt.bfloat16, "wl1_sb")
            wr1_sb = CONST(wr1p, [F, D1], dt.bfloat16, "wr1_sb")
            we1_sb = CONST(we1p, [F, D1], dt.bfloat16, "we1_sb")
            sr1_sb = CONST(srep1, [P, D1], dt.bfloat16, "sr1_sb")
            ainv_sb = CONST(attinv, [P, D1], dt.float32, "ainv_sb")
            wlr2_sb = CONST(wlr2, [P, 16 * 2 * D2], dt.bfloat16, "wlr2_sb")
            we2_sb = CONST(we2p, [F, D2], dt.bfloat16, "we2_sb")
            sr2_sb = CONST(srep2, [P, D2], dt.bfloat16, "sr2_sb")
            ainv2_sb = CONST(attinv2, [P, D2], dt.float32, "ainv2_sb")
            id_sb = CONST(ident, [P, P], dt.bfloat16, "id_sb")
            xr2all = cp.tile([P, G * D2], dt.bfloat16)

            # ---------------- phase B ----------------
            with (
                tc.tile_pool(name="b_acc", bufs=1, space="PSUM") as accp,
                tc.tile_pool(name="b_q", bufs=3, space="PSUM") as qp,
                tc.tile_pool(name="b_qs", bufs=2) as qsp,
                tc.tile_pool(name="b_s01", bufs=2) as s01p,
                tc.tile_pool(name="b_wk", bufs=3) as wk,
                tc.tile_pool(name="b_fin", bufs=2) as fnp,
            ):
                state = {"qs": None, "s01": None, "acc": None, "next": None}
                pend = {}
                finq = {}

                def load_streams(g):
                    qs_t = qsp.tile([P, CH * 384], dt.bfloat16, tag="qs", name="qs_t")
                    nc.sync.dma_start(
                        out=qs_t[:],
                        in_=qstream[:, g * CH * 384:(g + 1) * CH * 384])
                    s01_t = s01p.tile([P, CH * P], dt.bfloat16, tag="s01", name="s01_t")
                    nc.sync.dma_start(
                        out=s01_t[:],
                        in_=s01s[:, g * CH * P:(g + 1) * CH * P])
                    state["next"] = (qs_t, s01_t)

                def q_work(t):
                    g, c = divmod(t, CH)
                    if c == 0:
                        state["qs"], state["s01"] = state["next"]
                    if c == CH - 2 and g + 1 < G:
                        load_streams(g + 1)
                    qs_t = state["qs"]
                    o = c * 384
                    u_sb = wk.tile([P, D1], dt.bfloat16, tag="u")
                    xl_sb = wk.tile([P, D1], dt.bfloat16, tag="xl")
                    v_sb = wk.tile([P, D1], dt.bfloat16, tag="v")
                    lg = wk.tile([P, H1], dt.float32, tag="lg")
                    for qq in range(4):
                        qt = qp.tile([P, 512], dt.float32, tag="q")
                        nc.tensor.matmul(
                            out=qt[:], lhsT=qs_t[:, o:o + P],
                            rhs=wl1_sb[:, qq * 512:(qq + 1) * 512],
                            start=True, stop=False)
                        if qq % 2 == 0:
                            nc.vector.tensor_copy(
                                out=xl_sb[:, qq * 512:(qq + 1) * 512], in_=qt[:])
                        else:
                            nc.scalar.copy(
                                out=xl_sb[:, qq * 512:(qq + 1) * 512], in_=qt[:])
                        nc.tensor.matmul(
                            out=qt[:], lhsT=qs_t[:, o + P:o + 2 * P],
                            rhs=wr1_sb[:, qq * 512:(qq + 1) * 512],
                            start=False, stop=False)
                        nc.tensor.matmul(
                            out=qt[:], lhsT=qs_t[:, o + 2 * P:o + 3 * P],
                            rhs=we1_sb[:, qq * 512:(qq + 1) * 512],
                            start=False, stop=True)
                        nc.scalar.activation(
                            out=u_sb[:, qq * 512:(qq + 1) * 512], in_=qt[:],
                            func=LRELU, alpha=NEG)
                        for hh in range(2):
                            h = qq * 2 + hh
                            nc.vector.scalar_tensor_tensor(
                                out=v_sb[:, h * C1:(h + 1) * C1],
                                in0=u_sb[:, h * C1:(h + 1) * C1],
                                scalar=1.0,
                                in1=sr1_sb[:, h * C1:(h + 1) * C1],
                                op0=ALU.mult, op1=ALU.mult,
                                accum_out=lg[:, h:h + 1])
                    ex_f = wk.tile([P, H1], dt.float32, tag="exf")
                    nc.scalar.activation(out=ex_f[:], in_=lg[:], func=AF.Exp)
                    exw = wk.tile([P, H1], dt.bfloat16, tag="exw")
                    nc.vector.tensor_copy(out=exw[:], in_=ex_f[:])
                    xls = wk.tile([P, D1], dt.bfloat16, tag="xls")
                    nc.gpsimd.tensor_tensor(
                        out=xls[:].rearrange("p (h c) -> p h c", h=H1),
                        in0=xl_sb[:].rearrange("p (h c) -> p h c", h=H1),
                        in1=ex_f[:].rearrange("p (h c) -> p h c", c=1)
                            .broadcast_to([P, H1, C1]),
                        op=ALU.mult)
                    pend[t] = (state["s01"], xls, exw)

                def scatter(t):
                    g, c = divmod(t, CH)
                    if c == 0:
                        state["acc"] = accp.tile([P, D1 + 8], dt.float32, tag="acc", name="acc_t")
                    acc = state["acc"]
                    s01_t, xls, exw = pend.pop(t)
                    lhsT = s01_t[:, c * P:(c + 1) * P]
                    for j in range(4):
                        nc.tensor.matmul(
                            out=acc[:, j * 512:(j + 1) * 512], lhsT=lhsT,
                            rhs=xls[:, j * 512:(j + 1) * 512],
                            start=(c == 0), stop=(c == CH - 1))
                    nc.tensor.matmul(
                        out=acc[:, D1:D1 + 8], lhsT=lhsT, rhs=exw[:],
                        start=(c == 0), stop=(c == CH - 1))

                def finalizeA(g):
                    acc = state["acc"]
                    dn = fnp.tile([P, H1], dt.float32, tag="dn")
                    nc.vector.tensor_scalar(
                        out=dn[:], in0=acc[:, D1:D1 + 8], scalar1=1.0,
                        scalar2=1e-30, op0=ALU.mult, op1=ALU.max)
                    dnr = fnp.tile([P, H1], dt.float32, tag="dnr")
                    nc.vector.reciprocal(out=dnr[:], in_=dn[:])
                    hu = fnp.tile([P, D1], dt.bfloat16, tag="hu")
                    nc.vector.tensor_tensor(
                        out=hu[:], in0=acc[:, :D1], in1=ainv_sb[:], op=ALU.mult)
                    h_sb = fnp.tile([P, D1], dt.bfloat16, tag="h")
                    for h in range(H1):
                        nc.vector.tensor_scalar(
                            out=h_sb[:, h * C1:(h + 1) * C1],
                            in0=hu[:, h * C1:(h + 1) * C1],
                            scalar1=dnr[:, h:h + 1], scalar2=0.0,
                            op0=ALU.mult, op1=ALU.max)
                    hT = fnp.tile([P, D1], dt.bfloat16, tag="hT")
                    for kk in range(16):
                        nc.sync.dma_start(out=hT[:, kk * P:(kk + 1) * P],
                                          in_=h_sb[:, kk * P:(kk + 1) * P],
                                          transpose=True)
                    finq[g] = hT

                def finalizeB(g):
                    hT = finq.pop(g)
                    fin = qp.tile([P, 512], dt.float32, tag="q")
                    for kk in range(16):
                        nc.tensor.matmul(
                            out=fin[:, 0:2 * D2], lhsT=hT[:, kk * P:(kk + 1) * P],
                            rhs=wlr2_sb[:, kk * 2 * D2:(kk + 1) * 2 * D2],
                            start=(kk == 0), stop=(kk == 15))
                    xlr2_sb = fnp.tile([P, 2 * D2], dt.bfloat16, tag="xlr2")
                    nc.vector.tensor_copy(out=xlr2_sb[:], in_=fin[:, 0:2 * D2])
                    nc.vector.tensor_copy(out=xr2all[:, g * D2:(g + 1) * D2],
                                          in_=xlr2_sb[:, D2:2 * D2])
                    nc.sync.dma_start(out=xlr2_d[g * GN:(g + 1) * GN, :],
                                      in_=xlr2_sb[:GN])

                load_streams(0)
                for t in range(TC):
                    q_work(t)
                    if t >= 1:
                        scatter(t - 1)
                        if t % CH == 0:
                            finalizeA(t // CH - 1)
                    g, c = divmod(t, CH)
                    if c == FINC and g >= 1:
                        finalizeB(g - 1)
                scatter(TC - 1)
                finalizeA(G - 1)
                finalizeB(G - 1)

            # ---------------- AllGather ----------------
            nc.gpsimd.collective_compute(
                "AllGather", ALU.bypass, replica_groups=[list(range(M))],
                ins=[xlr2_d[:]], outs=[ag_out[:]])
            nc.sync.dma_start(out=xl2t[:], in_=ag_out[:, 0:D2])

            # ---------------- phase C ----------------
            with (
                tc.tile_pool(name="c_acc", bufs=1, space="PSUM") as cacc,
                tc.tile_pool(name="c_q", bufs=2, space="PSUM") as cq,
                tc.tile_pool(name="c_s01", bufs=2) as cs01,
                tc.tile_pool(name="c_s01T", bufs=2) as cs01T,
                tc.tile_pool(name="c_ea", bufs=2) as cea,
                tc.tile_pool(name="c_ge", bufs=CPRE + 2) as cge,
                tc.tile_pool(name="c_wk", bufs=3) as cw,
            ):
                cstate = {"s01": None, "s01T": None, "ea": None, "acc2": None, "next": None}
                xes = {}
                pend2 = {}

                def c_load(t):
                    si = cge.tile([P, 1], dt.int32, tag="si", name="si_t")
                    nc.sync.dma_start(out=si[:], in_=srcis[t * P:(t + 1) * P, :])
                    xe = cge.tile([P, D2], dt.bfloat16, tag="xe", name="xe_t")
                    nc.gpsimd.indirect_dma_start(
                        out=xe[:], out_offset=None, in_=xl2t[:],
                        in_offset=bass.IndirectOffsetOnAxis(ap=si[:, :1], axis=0))
                    xes[t] = xe

                def c_load_streams(g):
                    s01c_t = cs01.tile([P, CH * P], dt.bfloat16, tag="s01c", name="s01c_t")
                    nc.sync.dma_start(
                        out=s01c_t[:],
                        in_=s01s[:, g * CH * P:(g + 1) * CH * P])
                    s01Tc_t = cs01T.tile([P, CH * P], dt.bfloat16, tag="s01Tc", name="s01Tc_t")
                    nc.sync.dma_start(
                        out=s01Tc_t[:],
                        in_=s01Ts[:, g * CH * P:(g + 1) * CH * P])
                    eaTc_t = cea.tile([P, CH * P], dt.bfloat16, tag="eaTc", name="eaTc_t")
                    qsl = qstream[:, g * CH * 384:(g + 1) * CH * 384]
                    nc.sync.dma_start(
                        out=eaTc_t[:].rearrange("p (c w) -> p c w", c=CH),
                        in_=qsl.rearrange("p (c w) -> p c w", c=CH)[:, :, 256:384])
                    cstate["next"] = (s01c_t, s01Tc_t, eaTc_t)

                def c_work(t):
                    g, c = divmod(t, CH)
                    if c == 0:
                        cstate["s01"], cstate["s01T"], cstate["ea"] = cstate["next"]
                    if c == CH - 2 and g + 1 < G:
                        c_load_streams(g + 1)
                    xe = xes.pop(t)
                    q2 = cq.tile([P, D2], dt.float32, tag="q2")
                    nc.tensor.matmul(
                        out=q2[:], lhsT=cstate["s01T"][:, c * P:(c + 1) * P],
                        rhs=xr2all[:, g * D2:(g + 1) * D2], start=True, stop=False)
                    nc.tensor.matmul(
                        out=q2[:], lhsT=cstate["ea"][:, c * P:(c + 1) * P],
                        rhs=we2_sb[:], start=False, stop=False)
                    nc.tensor.matmul(
                        out=q2[:], lhsT=id_sb[:], rhs=xe[:], start=False, stop=True)
                    u2 = cw.tile([P, D2], dt.bfloat16, tag="u2")
                    nc.scalar.activation(out=u2[:], in_=q2[:], func=LRELU, alpha=NEG)
                    v2 = cw.tile([P, D2], dt.bfloat16, tag="v2")
                    lg2 = cw.tile([P, 1], dt.float32, tag="lg2")
                    nc.vector.scalar_tensor_tensor(
                        out=v2[:], in0=u2[:], scalar=1.0, in1=sr2_sb[:],
                        op0=ALU.mult, op1=ALU.mult, accum_out=lg2[:, :1])
                    ex2 = cw.tile([P, 1], dt.float32, tag="ex2")
                    nc.scalar.activation(out=ex2[:], in_=lg2[:], func=AF.Exp)
                    xls2 = cw.tile([P, D2 + 1], dt.bfloat16, tag="xls2")
                    nc.vector.tensor_scalar(
                        out=xls2[:, :D2], in0=xe[:], scalar1=ex2[:, :1],
                        scalar2=None, op0=ALU.mult)
                    nc.vector.tensor_copy(out=xls2[:, D2:], in_=ex2[:])
                    pend2[t] = (cstate["s01"], xls2)

                def scatter2(t):
                    g, c = divmod(t, CH)
                    if c == 0:
                        cstate["acc2"] = cacc.tile([P, D2 + 1], dt.float32, tag="acc2", name="acc2_t")
                    s01_t, xls2 = pend2.pop(t)
                    nc.tensor.matmul(
                        out=cstate["acc2"][:], lhsT=s01_t[:, c * P:(c + 1) * P],
                        rhs=xls2[:], start=(c == 0), stop=(c == CH - 1))

                def finalizeC(g):
                    acc2 = cstate["acc2"]
                    d2 = cw.tile([P, 1], dt.float32, tag="d2")
                    nc.vector.tensor_scalar(
                        out=d2[:], in0=acc2[:, D2:D2 + 1], scalar1=1.0,
                        scalar2=1e-30, op0=ALU.mult, op1=ALU.max)
                    d2r = cw.tile([P, 1], dt.float32, tag="d2r")
                    nc.vector.reciprocal(out=d2r[:], in_=d2[:])
                    o2u = cw.tile([P, D2], dt.float32, tag="o2u")
                    nc.vector.tensor_tensor(
                        out=o2u[:], in0=acc2[:, :D2], in1=ainv2_sb[:], op=ALU.mult)
                    o2 = cw.tile([P, D2], dt.float32, tag="o2")
                    nc.vector.tensor_scalar(
                        out=o2[:], in0=o2u[:], scalar1=d2r[:, :1], scalar2=0.0,
                        op0=ALU.mult, op1=ALU.max)
                    nc.sync.dma_start(out=out[g * GN:(g + 1) * GN, :], in_=o2[:GN])

                c_load_streams(0)
                for t in range(min(CPRE, TC)):
                    c_load(t)
                for t in range(TC):
                    if t + CPRE < TC:
                        c_load(t + CPRE)
                    c_work(t)
                    if t >= 1:
                        scatter2(t - 1)
                        if t % CH == 0:
                            finalizeC(t // CH - 1)
                scatter2(TC - 1)
                finalizeC(G - 1)

    nc.compile()
    return nc


def _prep_inputs(x, edge_index, edge_attr, Wl1, bl1, Wr1, br1, We1, att1, b1,
                 Wl2, bl2, Wr2, br2, We2, att2, b2):
    for b in (bl1, br1, b1, bl2, br2, b2):
        assert not np.any(np.asarray(b)), "nonzero biases not implemented"

    x = np.asarray(x, dtype=np.float32)
    src = np.asarray(edge_index[0], dtype=np.int64)
    dst = np.asarray(edge_index[1], dtype=np.int64)
    ea = np.asarray(edge_attr, dtype=np.float32)

    sums = np.zeros((N, F), np.float32)
    np.add.at(sums, dst, ea)
    cnt = np.bincount(dst, minlength=N).astype(np.float32)
    self_attr = sums / np.maximum(cnt, 1.0)[:, None]
    src_a = np.concatenate([src, np.arange(N, dtype=np.int64)])
    dst_a = np.concatenate([dst, np.arange(N, dtype=np.int64)])
    ea_a = np.concatenate([ea, self_attr], axis=0)

    order = np.argsort(dst_a, kind="stable")
    s_src, s_dst, s_ea = src_a[order], dst_a[order], ea_a[order]

    bounds = np.searchsorted(s_dst, np.arange(0, N + GN, GN))
    cnts = np.diff(bounds)
    CH = int(np.max((cnts + P - 1) // P))
    L = G * CH * P

    # att folding: per-head pos-att-first column permutation, |att| into W
    A1 = np.asarray(att1, np.float32).reshape(H1, C1)
    perm1 = np.zeros(D1, np.int64)
    for h in range(H1):
        pos = np.nonzero(A1[h] >= 0)[0]
        neg = np.nonzero(A1[h] < 0)[0]
        perm1[h * C1:(h + 1) * C1] = h * C1 + np.concatenate([pos, neg])
    A1f = A1.reshape(-1)[perm1]
    assert np.abs(A1f).min() > 1e-7
    s1 = np.sign(A1f).astype(np.float32)
    a1 = np.abs(A1f)

    A2 = np.asarray(att2, np.float32).reshape(-1)
    perm2 = np.concatenate([np.nonzero(A2 >= 0)[0], np.nonzero(A2 < 0)[0]])
    A2f = A2[perm2]
    assert np.abs(A2f).min() > 1e-7
    s2 = np.sign(A2f).astype(np.float32)
    a2 = np.abs(A2f)

    wl1p = (np.asarray(Wl1, np.float32)[:, perm1] * a1).astype(BF16)
    wr1p = (np.asarray(Wr1, np.float32)[:, perm1] * a1).astype(BF16)
    we1p = (np.asarray(We1, np.float32)[:, perm1] * a1).astype(BF16)
    srep1 = np.tile(s1, (P, 1)).astype(BF16)
    attinv = np.tile((1.0 / a1).astype(np.float32), (P, 1))

    Wl2pp = np.asarray(Wl2, np.float32)[perm1][:, perm2] * a2
    Wr2pp = np.asarray(Wr2, np.float32)[perm1][:, perm2] * a2
    wlr2 = np.zeros((P, 16 * 2 * D2), np.float32)
    for kk in range(16):
        wlr2[:, kk * 2 * D2:kk * 2 * D2 + D2] = Wl2pp[kk * P:(kk + 1) * P]
        wlr2[:, kk * 2 * D2 + D2:(kk + 1) * 2 * D2] = Wr2pp[kk * P:(kk + 1) * P]
    we2p = (np.asarray(We2, np.float32)[:, perm2] * a2).astype(BF16)
    srep2 = np.tile(s2, (P, 1)).astype(BF16)
    attinv2 = np.tile((1.0 / a2).astype(np.float32), (P, 1))

    common = {
        "wl1p": wl1p, "wr1p": wr1p, "we1p": we1p, "srep1": srep1,
        "attinv": attinv, "wlr2": wlr2.astype(BF16), "we2p": we2p,
        "srep2": srep2, "attinv2": attinv2,
        "ident": np.eye(P, dtype=np.float32).astype(BF16),
    }

    in_maps = []
    for k in range(M):
        base_node = k * NPC
        qstream = np.zeros((P, G * CH * 384), np.float32)
        s01s = np.zeros((P, G * CH * P), np.float32)
        s01Ts = np.zeros((P, G * CH * P), np.float32)
        srcis = np.zeros((L, 1), np.int32)
        for g in range(G):
            gb = base_node + g * GN
            lo, hi = bounds[k * G + g], bounds[k * G + g + 1]
            cnt_g = hi - lo
            gsrc = s_src[lo:hi]
            gdst = s_dst[lo:hi]
            gea = s_ea[lo:hi]
            dl = (gdst - gb).astype(np.int64)
            for c in range(CH):
                e0 = c * P
                n_e = min(P, cnt_g - e0) if cnt_g > e0 else 0
                o = (g * CH + c) * 384
                so = (g * CH + c) * P
                if n_e > 0:
                    sl = slice(e0, e0 + n_e)
                    qstream[:, o:o + n_e] = x[gsrc[sl]].T
                    qstream[:, o + P:o + P + n_e] = x[gdst[sl]].T
                    qstream[:, o + 2 * P:o + 2 * P + n_e] = gea[sl].T
                    ee = np.arange(n_e)
                    s01s[ee, so + dl[sl]] = 1.0
                    s01Ts[dl[sl], so + ee] = 1.0
                    srcis[so:so + n_e, 0] = gsrc[sl]
        im = dict(common)
        im["qstream"] = qstream.astype(BF16)
        im["s01s"] = s01s.astype(BF16)
        im["s01Ts"] = s01Ts.astype(BF16)
        im["srcis"] = srcis
        in_maps.append(im)
    return in_maps, CH, perm2


_PROG_CACHE = {}


def _get_program(CH):
    if CH not in _PROG_CACHE:
        _PROG_CACHE[CH] = _build_program(CH)
    return _PROG_CACHE[CH]


def run(inputs, trace=False, tmpdir=None):
    in_maps, CH, perm2 = _prep_inputs(**inputs)
    nc = _get_program(CH)
    res = run_bass_kernel_spmd(nc, in_maps, list(range(M)), trace=trace,
                               tmpdir=tmpdir)
    outp = np.concatenate([res.results[k]["out"] for k in range(M)], axis=0)
    final = np.empty_like(outp)
    final[:, perm2] = outp
    return final.astype(np.float32), res


def kernel(**inputs):
    outp, _ = run(inputs)
    return outp


# revision 3
# speedup vs baseline: 1.0387x; 1.0119x over previous
"""GATv2 2-layer GNN on 8 Trainium2 NeuronCores (Bass/Tile, edge-parallel v3).

Key structure:
- dst-sorted edges, dst-range sharded (core k owns nodes [1250k,1250(k+1))),
  10 groups x 125 dst nodes, CH chunks of 128 edges (self loops packed on
  host with mean edge_attr).
- Host pre-gathers x[src]^T | x[dst]^T | ea^T per chunk into one stream;
  layer-1 q accumulates fully in PSUM (3 matmul passes), xl extracted by a
  psum->sbuf copy between the passes.
- |att| folded into weight columns (host), columns permuted pos-att-first
  per head: logits = 3D-blockwise reduce of sign * Prelu(q), one scalar
  activation per 512-quarter, one gpsimd TT (sign), one vector reduce.
- alpha scaling of the aggregation undone at group finalize (1/|att| TT).
- scatter matmuls software-pipelined one chunk late; group finalize
  (transpose + layer-2 projection) deferred into the next group's chunks.
- layer 2 identical tricks at width 32; AllGather ships only [N,64] bf16.
"""
import sys
sys.path.insert(0, "/opt/trn_rl_repo")

import numpy as np
import ml_dtypes

import concourse.bass as bass
import concourse.bacc as bacc
import concourse.tile as tile
from concourse import mybir
from concourse.bass_utils import run_bass_kernel_spmd

BF16 = ml_dtypes.bfloat16

N, E, F = 10000, 80000, 128
H1, C1 = 8, 256
D1 = H1 * C1
D2 = 32
NEG = 0.2
M = 8
NPC = N // M
GN = 125
G = NPC // GN
P = 128

dt = mybir.dt
AF = mybir.ActivationFunctionType
ALU = mybir.AluOpType
LRELU = AF.Prelu
FINC = 6          # chunk index within next group at which deferred fin-MMs run
CPRE = 2          # phase-C gather prefetch depth


def _build_program(CH):
    L = G * CH * P
    TC = G * CH
    nc = bacc.Bacc("TRN2", target_bir_lowering=False, debug=False, num_devices=M)

    def EIN(name, shape, dtype):
        return nc.dram_tensor(name, list(shape), dtype, kind="ExternalInput")

    qstream = EIN("qstream", (P, G * CH * 384), dt.bfloat16)
    s01s    = EIN("s01s",    (P, G * CH * P),   dt.bfloat16)
    s01Ts   = EIN("s01Ts",   (P, G * CH * P),   dt.bfloat16)
    srcis   = EIN("srcis",   (L, 1),            dt.int32)
    wl1p    = EIN("wl1p",    (F, D1), dt.bfloat16)
    wr1p    = EIN("wr1p",    (F, D1), dt.bfloat16)
    we1p    = EIN("we1p",    (F, D1), dt.bfloat16)
    srep1   = EIN("srep1",   (P, D1), dt.bfloat16)
    attinv  = EIN("attinv",  (P, D1), dt.float32)
    wlr2    = EIN("wlr2",    (P, 16 * 2 * D2), dt.bfloat16)
    we2p    = EIN("we2p",    (F, D2), dt.bfloat16)
    srep2   = EIN("srep2",   (P, D2), dt.bfloat16)
    attinv2 = EIN("attinv2", (P, D2), dt.float32)
    ident   = EIN("ident",   (P, P), dt.bfloat16)

    out = nc.dram_tensor("out", [NPC, D2], dt.float32, kind="ExternalOutput")
    xlr2_d = nc.dram_tensor("xlr2_d", [NPC, 2 * D2], dt.bfloat16)
    ag_out = nc.dram_tensor("ag_out", [N, 2 * D2], dt.bfloat16, addr_space="Shared")
    xl2t   = nc.dram_tensor("xl2t", [N, D2], dt.bfloat16)

    with tile.TileContext(nc) as tc:
        with tc.tile_pool(name="consts", bufs=1) as cp:
            def CONST(dram, shape, dtype, nm):
                t = cp.tile(shape, dtype, name=nm, tag=nm)
                nc.sync.dma_start(out=t[:], in_=dram[:])
                return t
            wl1_sb = CONST(wl1p, [F, D1], dt.bfloat16, "wl1_sb")
            wr1_sb = CONST(wr1p, [F, D1], dt.bfloat16, "wr1_sb")
            we1_sb = CONST(we1p, [F, D1], dt.bfloat16, "we1_sb")
            sr1_sb = CONST(srep1, [P, D1], dt.bfloat16, "sr1_sb")
            ainv_sb = CONST(attinv, [P, D1], dt.float32, "ainv_sb")
            wlr2_sb = CONST(wlr2, [P, 16 * 2 * D2], dt.bfloat16, "wlr2_sb")
            we2_sb = CONST(we2p, [F, D2], dt.bfloat16, "we2_sb")
            sr2_sb = CONST(srep2, [P, D2], dt.bfloat16, "sr2_sb")
            ainv2_sb = CONST(attinv2, [P, D2], dt.float32, "ainv2_sb")
            id_sb = CONST(ident, [P, P], dt.bfloat16, "id_sb")
            xr2all = cp.tile([P, G * D2], dt.bfloat16)

            # ---------------- phase B ----------------
            with (
                tc.tile_pool(name="b_acc", bufs=1, space="PSUM") as accp,
                tc.tile_pool(name="b_q", bufs=3, space="PSUM") as qp,
                tc.tile_pool(name="b_qs", bufs=2) as qsp,
                tc.tile_pool(name="b_s01", bufs=2) as s01p,
                tc.tile_pool(name="b_wk", bufs=3) as wk,
                tc.tile_pool(name="b_fin", bufs=2) as fnp,
            ):
                state = {"qs": None, "s01": None, "acc": None, "next": None}
                pend = {}
                finq = {}

                def load_streams(g):
                    qs_t = qsp.tile([P, CH * 384], dt.bfloat16, tag="qs", name="qs_t")
                    nc.sync.dma_start(
                        out=qs_t[:],
                        in_=qstream[:, g * CH * 384:(g + 1) * CH * 384])
                    s01_t = s01p.tile([P, CH * P], dt.bfloat16, tag="s01", name="s01_t")
                    nc.sync.dma_start(
                        out=s01_t[:],
                        in_=s01s[:, g * CH * P:(g + 1) * CH * P])
                    state["next"] = (qs_t, s01_t)

                def q_work(t):
                    g, c = divmod(t, CH)
                    if c == 0:
                        state["qs"], state["s01"] = state["next"]
                    if c == CH - 2 and g + 1 < G:
                        load_streams(g + 1)
                    qs_t = state["qs"]
                    o = c * 384
                    u_sb = wk.tile([P, D1], dt.bfloat16, tag="u")
                    xl_sb = wk.tile([P, D1], dt.bfloat16, tag="xl")
                    v_sb = wk.tile([P, D1], dt.bfloat16, tag="v")
                    lg = wk.tile([P, H1], dt.float32, tag="lg")
                    for qq in range(4):
                        qt = qp.tile([P, 512], dt.float32, tag="q")
                        nc.tensor.matmul(
                            out=qt[:], lhsT=qs_t[:, o:o + P],
                            rhs=wl1_sb[:, qq * 512:(qq + 1) * 512],
                            start=True, stop=False)
                        if qq % 2 == 0:
                            nc.vector.tensor_copy(
                                out=xl_sb[:, qq * 512:(qq + 1) * 512], in_=qt[:])
                        else:
                            nc.scalar.copy(
                                out=xl_sb[:, qq * 512:(qq + 1) * 512], in_=qt[:])
                        nc.tensor.matmul(
                            out=qt[:], lhsT=qs_t[:, o + P:o + 2 * P],
                            rhs=wr1_sb[:, qq * 512:(qq + 1) * 512],
                            start=False, stop=False)
                        nc.tensor.matmul(
                            out=qt[:], lhsT=qs_t[:, o + 2 * P:o + 3 * P],
                            rhs=we1_sb[:, qq * 512:(qq + 1) * 512],
                            start=False, stop=True)
                        nc.scalar.activation(
                            out=u_sb[:, qq * 512:(qq + 1) * 512], in_=qt[:],
                            func=LRELU, alpha=NEG)
                        for hh in range(2):
                            h = qq * 2 + hh
                            nc.vector.scalar_tensor_tensor(
                                out=v_sb[:, h * C1:(h + 1) * C1],
                                in0=u_sb[:, h * C1:(h + 1) * C1],
                                scalar=1.0,
                                in1=sr1_sb[:, h * C1:(h + 1) * C1],
                                op0=ALU.mult, op1=ALU.mult,
                                accum_out=lg[:, h:h + 1])
                    exw = wk.tile([P, H1], dt.bfloat16, tag="exw")
                    nc.scalar.activation(out=exw[:], in_=lg[:], func=AF.Exp)
                    xls = wk.tile([P, D1], dt.bfloat16, tag="xls")
                    nc.gpsimd.tensor_tensor(
                        out=xls[:].rearrange("p (h c) -> p h c", h=H1),
                        in0=xl_sb[:].rearrange("p (h c) -> p h c", h=H1),
                        in1=exw[:].rearrange("p (h c) -> p h c", c=1)
                            .broadcast_to([P, H1, C1]),
                        op=ALU.mult)
                    pend[t] = (state["s01"], xls, exw)

                def scatter(t):
                    g, c = divmod(t, CH)
                    if c == 0:
                        state["acc"] = accp.tile([P, D1 + 8], dt.float32, tag="acc", name="acc_t")
                    acc = state["acc"]
                    s01_t, xls, exw = pend.pop(t)
                    lhsT = s01_t[:, c * P:(c + 1) * P]
                    for j in range(4):
                        nc.tensor.matmul(
                            out=acc[:, j * 512:(j + 1) * 512], lhsT=lhsT,
                            rhs=xls[:, j * 512:(j + 1) * 512],
                            start=(c == 0), stop=(c == CH - 1))
                    nc.tensor.matmul(
                        out=acc[:, D1:D1 + 8], lhsT=lhsT, rhs=exw[:],
                        start=(c == 0), stop=(c == CH - 1))

                def finalizeA(g):
                    acc = state["acc"]
                    dn = fnp.tile([P, H1], dt.float32, tag="dn")
                    nc.vector.tensor_scalar(
                        out=dn[:], in0=acc[:, D1:D1 + 8], scalar1=1.0,
                        scalar2=1e-30, op0=ALU.mult, op1=ALU.max)
                    dnr = fnp.tile([P, H1], dt.float32, tag="dnr")
                    nc.vector.reciprocal(out=dnr[:], in_=dn[:])
                    hu = fnp.tile([P, D1], dt.bfloat16, tag="hu")
                    nc.vector.tensor_tensor(
                        out=hu[:], in0=acc[:, :D1], in1=ainv_sb[:], op=ALU.mult)
                    h_sb = fnp.tile([P, D1], dt.bfloat16, tag="h")
                    for h in range(H1):
                        nc.vector.tensor_scalar(
                            out=h_sb[:, h * C1:(h + 1) * C1],
                            in0=hu[:, h * C1:(h + 1) * C1],
                            scalar1=dnr[:, h:h + 1], scalar2=0.0,
                            op0=ALU.mult, op1=ALU.max)
                    hT = fnp.tile([P, D1], dt.bfloat16, tag="hT")
                    for kk in range(16):
                        nc.sync.dma_start(out=hT[:, kk * P:(kk + 1) * P],
                                          in_=h_sb[:, kk * P:(kk + 1) * P],
                                          transpose=True)
                    finq[g] = hT

                def finalizeB(g):
                    hT = finq.pop(g)
                    fin = qp.tile([P, 512], dt.float32, tag="q")
                    for kk in range(16):
                        nc.tensor.matmul(
                            out=fin[:, 0:2 * D2], lhsT=hT[:, kk * P:(kk + 1) * P],
                            rhs=wlr2_sb[:, kk * 2 * D2:(kk + 1) * 2 * D2],
                            start=(kk == 0), stop=(kk == 15))
                    xlr2_sb = fnp.tile([P, 2 * D2], dt.bfloat16, tag="xlr2")
                    nc.vector.tensor_copy(out=xlr2_sb[:], in_=fin[:, 0:2 * D2])
                    nc.vector.tensor_copy(out=xr2all[:, g * D2:(g + 1) * D2],
                                          in_=xlr2_sb[:, D2:2 * D2])
                    nc.sync.dma_start(out=xlr2_d[g * GN:(g + 1) * GN, :],
                                      in_=xlr2_sb[:GN])

                load_streams(0)
                for t in range(TC):
                    q_work(t)
                    if t >= 1:
                        scatter(t - 1)
                        if t % CH == 0:
                            finalizeA(t // CH - 1)
                    g, c = divmod(t, CH)
                    if c == FINC and g >= 1:
                        finalizeB(g - 1)
                scatter(TC - 1)
                finalizeA(G - 1)
                finalizeB(G - 1)

            # ---------------- AllGather ----------------
            nc.gpsimd.collective_compute(
                "AllGather", ALU.bypass, replica_groups=[list(range(M))],
                ins=[xlr2_d[:]], outs=[ag_out[:]])
            nc.sync.dma_start(out=xl2t[:], in_=ag_out[:, 0:D2])

            # ---------------- phase C ----------------
            with (
                tc.tile_pool(name="c_acc", bufs=1, space="PSUM") as cacc,
                tc.tile_pool(name="c_q", bufs=2, space="PSUM") as cq,
                tc.tile_pool(name="c_s01", bufs=2) as cs01,
                tc.tile_pool(name="c_s01T", bufs=2) as cs01T,
                tc.tile_pool(name="c_ea", bufs=2) as cea,
                tc.tile_pool(name="c_ge", bufs=CPRE + 2) as cge,
                tc.tile_pool(name="c_wk", bufs=3) as cw,
            ):
                cstate = {"s01": None, "s01T": None, "ea": None, "acc2": None, "next": None}
                xes = {}
                pend2 = {}

                def c_load(t):
                    si = cge.tile([P, 1], dt.int32, tag="si", name="si_t")
                    nc.sync.dma_start(out=si[:], in_=srcis[t * P:(t + 1) * P, :])
                    xe = cge.tile([P, D2], dt.bfloat16, tag="xe", name="xe_t")
                    nc.gpsimd.indirect_dma_start(
                        out=xe[:], out_offset=None, in_=xl2t[:],
                        in_offset=bass.IndirectOffsetOnAxis(ap=si[:, :1], axis=0))
                    xes[t] = xe

                def c_load_streams(g):
                    s01c_t = cs01.tile([P, CH * P], dt.bfloat16, tag="s01c", name="s01c_t")
                    nc.sync.dma_start(
                        out=s01c_t[:],
                        in_=s01s[:, g * CH * P:(g + 1) * CH * P])
                    s01Tc_t = cs01T.tile([P, CH * P], dt.bfloat16, tag="s01Tc", name="s01Tc_t")
                    nc.sync.dma_start(
                        out=s01Tc_t[:],
                        in_=s01Ts[:, g * CH * P:(g + 1) * CH * P])
                    eaTc_t = cea.tile([P, CH * P], dt.bfloat16, tag="eaTc", name="eaTc_t")
                    qsl = qstream[:, g * CH * 384:(g + 1) * CH * 384]
                    nc.sync.dma_start(
                        out=eaTc_t[:].rearrange("p (c w) -> p c w", c=CH),
                        in_=qsl.rearrange("p (c w) -> p c w", c=CH)[:, :, 256:384])
                    cstate["next"] = (s01c_t, s01Tc_t, eaTc_t)

                def c_work(t):
                    g, c = divmod(t, CH)
                    if c == 0:
                        cstate["s01"], cstate["s01T"], cstate["ea"] = cstate["next"]
                    if c == CH - 2 and g + 1 < G:
                        c_load_streams(g + 1)
                    xe = xes.pop(t)
                    q2 = cq.tile([P, D2], dt.float32, tag="q2")
                    nc.tensor.matmul(
                        out=q2[:], lhsT=cstate["s01T"][:, c * P:(c + 1) * P],
                        rhs=xr2all[:, g * D2:(g + 1) * D2], start=True, stop=False)
                    nc.tensor.matmul(
                        out=q2[:], lhsT=cstate["ea"][:, c * P:(c + 1) * P],
                        rhs=we2_sb[:], start=False, stop=False)
                    nc.tensor.matmul(
                        out=q2[:], lhsT=id_sb[:], rhs=xe[:], start=False, stop=True)
                    u2 = cw.tile([P, D2], dt.bfloat16, tag="u2")
                    nc.scalar.activation(out=u2[:], in_=q2[:], func=LRELU, alpha=NEG)
                    v2 = cw.tile([P, D2], dt.bfloat16, tag="v2")
                    lg2 = cw.tile([P, 1], dt.float32, tag="lg2")
                    nc.vector.scalar_tensor_tensor(
                        out=v2[:], in0=u2[:], scalar=1.0, in1=sr2_sb[:],
                        op0=ALU.mult, op1=ALU.mult, accum_out=lg2[:, :1])
                    ex2 = cw.tile([P, 1], dt.float32, tag="ex2")
                    nc.scalar.activation(out=ex2[:], in_=lg2[:], func=AF.Exp)
                    xls2 = cw.tile([P, D2 + 1], dt.bfloat16, tag="xls2")
                    nc.vector.tensor_scalar(
                        out=xls2[:, :D2], in0=xe[:], scalar1=ex2[:, :1],
                        scalar2=None, op0=ALU.mult)
                    nc.vector.tensor_copy(out=xls2[:, D2:], in_=ex2[:])
                    pend2[t] = (cstate["s01"], xls2)

                def scatter2(t):
                    g, c = divmod(t, CH)
                    if c == 0:
                        cstate["acc2"] = cacc.tile([P, D2 + 1], dt.float32, tag="acc2", name="acc2_t")
                    s01_t, xls2 = pend2.pop(t)
                    nc.tensor.matmul(
                        out=cstate["acc2"][:], lhsT=s01_t[:, c * P:(c + 1) * P],
                        rhs=xls2[:], start=(c == 0), stop=(c == CH - 1))

                def finalizeC(g):
                    acc2 = cstate["acc2"]
                    d2 = cw.tile([P, 1], dt.float32, tag="d2")
                    nc.vector.tensor_scalar(
                        out=d2[:], in0=acc2[:, D2:D2 + 1], scalar1=1.0,
                        scalar2=1e-30, op0=ALU.mult, op1=ALU.max)
                    d2r = cw.tile([P, 1], dt.float32, tag="d2r")
                    nc.vector.reciprocal(out=d2r[:], in_=d2[:])
                    o2u = cw.tile([P, D2], dt.float32, tag="o2u")
                    nc.vector.tensor_tensor(
                        out=o2u[:], in0=acc2[:, :D2], in1=ainv2_sb[:], op=ALU.mult)
                    o2 = cw.tile([P, D2], dt.float32, tag="o2")
                    nc.vector.tensor_scalar(
                        out=o2[:], in0=o2u[:], scalar1=d2r[:, :1], scalar2=0.0,
                        op0=ALU.mult, op1=ALU.max)
                    nc.sync.dma_start(out=out[g * GN:(g + 1) * GN, :], in_=o2[:GN])

                c_load_streams(0)
                for t in range(min(CPRE, TC)):
                    c_load(t)
                for t in range(TC):
                    if t + CPRE < TC:
                        c_load(t + CPRE)
                    c_work(t)
                    if t >= 1:
                        scatter2(t - 1)
                        if t % CH == 0:
                            finalizeC(t // CH - 1)
                scatter2(TC - 1)
                finalizeC(G - 1)

    nc.compile()
    return nc


def _prep_inputs(x, edge_index, edge_attr, Wl1, bl1, Wr1, br1, We1, att1, b1,
                 Wl2, bl2, Wr2, br2, We2, att2, b2):
    for b in (bl1, br1, b1, bl2, br2, b2):
        assert not np.any(np.asarray(b)), "nonzero biases not implemented"

    x = np.asarray(x, dtype=np.float32)
    src = np.asarray(edge_index[0], dtype=np.int64)
    dst = np.asarray(edge_index[1], dtype=np.int64)
    ea = np.asarray(edge_attr, dtype=np.float32)

    sums = np.zeros((N, F), np.float32)
    np.add.at(sums, dst, ea)
    cnt = np.bincount(dst, minlength=N).astype(np.float32)
    self_attr = sums / np.maximum(cnt, 1.0)[:, None]
    src_a = np.concatenate([src, np.arange(N, dtype=np.int64)])
    dst_a = np.concatenate([dst, np.arange(N, dtype=np.int64)])
    ea_a = np.concatenate([ea, self_attr], axis=0)

    order = np.argsort(dst_a, kind="stable")
    s_src, s_dst, s_ea = src_a[order], dst_a[order], ea_a[order]

    bounds = np.searchsorted(s_dst, np.arange(0, N + GN, GN))
    cnts = np.diff(bounds)
    CH = int(np.max((cnts + P - 1) // P))
    L = G * CH * P

    # att folding: per-head pos-att-first column permutation, |att| into W
    A1 = np.asarray(att1, np.float32).reshape(H1, C1)
    perm1 = np.zeros(D1, np.int64)
    for h in range(H1):
        pos = np.nonzero(A1[h] >= 0)[0]
        neg = np.nonzero(A1[h] < 0)[0]
        perm1[h * C1:(h + 1) * C1] = h * C1 + np.concatenate([pos, neg])
    A1f = A1.reshape(-1)[perm1]
    assert np.abs(A1f).min() > 1e-7
    s1 = np.sign(A1f).astype(np.float32)
    a1 = np.abs(A1f)

    A2 = np.asarray(att2, np.float32).reshape(-1)
    perm2 = np.concatenate([np.nonzero(A2 >= 0)[0], np.nonzero(A2 < 0)[0]])
    A2f = A2[perm2]
    assert np.abs(A2f).min() > 1e-7
    s2 = np.sign(A2f).astype(np.float32)
    a2 = np.abs(A2f)

    wl1p = (np.asarray(Wl1, np.float32)[:, perm1] * a1).astype(BF16)
    wr1p = (np.asarray(Wr1, np.float32)[:, perm1] * a1).astype(BF16)
    we1p = (np.asarray(We1, np.float32)[:, perm1] * a1).astype(BF16)
    srep1 = np.tile(s1, (P, 1)).astype(BF16)
    attinv = np.tile((1.0 / a1).astype(np.float32), (P, 1))

    Wl2pp = np.asarray(Wl2, np.float32)[perm1][:, perm2] * a2
    Wr2pp = np.asarray(Wr2, np.float32)[perm1][:, perm2] * a2
    wlr2 = np.zeros((P, 16 * 2 * D2), np.float32)
    for kk in range(16):
        wlr2[:, kk * 2 * D2:kk * 2 * D2 + D2] = Wl2pp[kk * P:(kk + 1) * P]
        wlr2[:, kk * 2 * D2 + D2:(kk + 1) * 2 * D2] = Wr2pp[kk * P:(kk + 1) * P]
    we2p = (np.asarray(We2, np.float32)[:, perm2] * a2).astype(BF16)
    srep2 = np.tile(s2, (P, 1)).astype(BF16)
    attinv2 = np.tile((1.0 / a2).astype(np.float32), (P, 1))

    common = {
        "wl1p": wl1p, "wr1p": wr1p, "we1p": we1p, "srep1": srep1,
        "attinv": attinv, "wlr2": wlr2.astype(BF16), "we2p": we2p,
        "srep2": srep2, "attinv2": attinv2,
        "ident": np.eye(P, dtype=np.float32).astype(BF16),
    }

    in_maps = []
    for k in range(M):
        base_node = k * NPC
        qstream = np.zeros((P, G * CH * 384), np.float32)
        s01s = np.zeros((P, G * CH * P), np.float32)
        s01Ts = np.zeros((P, G * CH * P), np.float32)
        srcis = np.zeros((L, 1), np.int32)
        for g in range(G):
            gb = base_node + g * GN
            lo, hi = bounds[k * G + g], bounds[k * G + g + 1]
            cnt_g = hi - lo
            gsrc = s_src[lo:hi]
            gdst = s_dst[lo:hi]
            gea = s_ea[lo:hi]
            dl = (gdst - gb).astype(np.int64)
            for c in range(CH):
                e0 = c * P
                n_e = min(P, cnt_g - e0) if cnt_g > e0 else 0
                o = (g * CH + c) * 384
                so = (g * CH + c) * P
                if n_e > 0:
                    sl = slice(e0, e0 + n_e)
                    qstream[:, o:o + n_e] = x[gsrc[sl]].T
                    qstream[:, o + P:o + P + n_e] = x[gdst[sl]].T
                    qstream[:, o + 2 * P:o + 2 * P + n_e] = gea[sl].T
                    ee = np.arange(n_e)
                    s01s[ee, so + dl[sl]] = 1.0
                    s01Ts[dl[sl], so + ee] = 1.0
                    srcis[so:so + n_e, 0] = gsrc[sl]
        im = dict(common)
        im["qstream"] = qstream.astype(BF16)
        im["s01s"] = s01s.astype(BF16)
        im["s01Ts"] = s01Ts.astype(BF16)
        im["srcis"] = srcis
        in_maps.append(im)
    return in_maps, CH, perm2


_PROG_CACHE = {}


def _get_program(CH):
    if CH not in _PROG_CACHE:
        _PROG_CACHE[CH] = _build_program(CH)
    return _PROG_CACHE[CH]


def run(inputs, trace=False, tmpdir=None):
    in_maps, CH, perm2 = _prep_inputs(**inputs)
    nc = _get_program(CH)
    res = run_bass_kernel_spmd(nc, in_maps, list(range(M)), trace=trace,
                               tmpdir=tmpdir)
    outp = np.concatenate([res.results[k]["out"] for k in range(M)], axis=0)
    final = np.empty_like(outp)
    final[:, perm2] = outp
    return final.astype(np.float32), res


def kernel(**inputs):
    outp, _ = run(inputs)
    return outp
